# revision 47
# baseline (speedup 1.0000x reference)
"""AugmentedLSTMCell on 8 TRN2 NeuronCores — data-parallel over batch.

Layout: feature-on-partition (transposed). Per core: B_loc=2048 batch rows.
  proj.T[j, b] = sum_e W[j, e] * in[b, e]
  lhsT tiles  = W.T blocks [128e, 128j]  (host pre-packed, bf16)
  rhs         = in.T        [128e, 2048b] (host pre-transposed, bf16)
  psum [128j, 2048b] accumulates the Wi-proj and Ws-proj contraction
  (the "fused = proj_in + proj_st" add comes free via PSUM accumulation).
  ScalarE applies per-feature bias + sigmoid/tanh straight out of PSUM.
Host transposes outputs back to [B, H].

Perf structure:
  - partial fp8: the first 2 of 8 contraction k-tiles of every GATE
    projection run as one DoubleRow fp8 matmul (2 k-tiles per
    instruction at ~1.4-1.8x bf16 throughput). hwp (the highway
    projection, which enters the output linearly) stays full bf16, as
    do k-tiles 2-7 of the gates, so the added quantization error is
    ~sqrt(0.25) of full-fp8 -> rel_err ~1.3e-2 (limit 2e-2).
    fp8 product scale S = sW*sA is folded out via the activation's
    scale operand; the bf16 gate weights are pre-scaled by S on host.
  - outputs written as bf16 (halves output HBM traffic; host upcasts)
  - last tile group reordered: hwp computed LAST in per-bc chunks with
    fused blend+DMA so the post-matmul tail is ~3us instead of ~7us
"""
import sys
import types

sys.path.insert(0, "/opt/trn_rl_repo")
sys.path.insert(0, "/root/.axon_site")

# Shim antenv.axon_hooks (missing on this image) so trace=True can profile.
if "antenv.axon_hooks" not in sys.modules:
    _hooks = types.ModuleType("antenv.axon_hooks")
    _state = {"hook": None}
    _hooks.set_axon_ntff_profile_hook = lambda h: _state.__setitem__("hook", h)
    _hooks.get_axon_ntff_profile_hook = lambda: _state["hook"]
    sys.modules["antenv.axon_hooks"] = _hooks
    try:
        from trn_agent_boot.trn_boot import _ntff_profile_via_ctypes

        _hooks.set_axon_ntff_profile_hook(
            _ntff_profile_via_ctypes("/opt/axon/libaxon_pjrt.so")
        )
    except Exception:
        pass

import numpy as np
import ml_dtypes

import concourse.bass as bass
import concourse.bacc as bacc
import concourse.mybir as mybir
from concourse import tile
from concourse.bass_utils import run_bass_kernel_spmd

BF16 = ml_dtypes.bfloat16
F8E4 = ml_dtypes.float8_e4m3fn

N_CORES = 8
B, E, H = 16384, 1024, 1024
BL = B // N_CORES          # 2048 batch rows per core
KT = E // 128              # 8 contraction k-tiles
KF = 2                     # minimum fp8 DoubleRow k-tiles (the m gate)
KF4 = 4                    # fp8 k-tiles for i/f gates


def gate_kf(jt):
    """fp8 k-tiles per gate tile: m->2 (dominates mem error), i/f->6,
    o/hw->8 (their error reaches `out` only via sigmoid-compressed paths)."""
    if 2 * NT <= jt < 3 * NT:
        return 2          # m
    if jt >= 3 * NT:
        return 8          # o, hw
    return 6              # i, f
NJI = 6 * H // 128         # 48 feature tiles of proj_in
NJS = 5 * H // 128         # 40 feature tiles of proj_st (the gates)
NT = H // 128              # 8 H-slices
BC = 512                   # matmul moving free dim (one PSUM bank)
NBC = BL // BC             # batch chunks per matmul group
KB = KT - KF               # bf16 k-tiles per gate side

USE_FP8 = True

AF = mybir.ActivationFunctionType
DR = mybir.MatmulPerfMode.DoubleRow


def build_nc():
    nc = bacc.Bacc(None, target_bir_lowering=False)
    f32, bf16 = mybir.dt.float32, mybir.dt.bfloat16
    f8 = mybir.dt.float8e4

    xT = nc.declare_dram_parameter("xT", [E, BL], bf16, isOutput=False)
    hT = nc.declare_dram_parameter("hT", [H, BL], bf16, isOutput=False)
    cT = nc.declare_dram_parameter("cT", [H, BL], bf16, isOutput=False)
    whwp = nc.declare_dram_parameter("whwp", [NT, 128, E], bf16, isOutput=False)
    bias = nc.declare_dram_parameter("bias", [128, NJI], f32, isOutput=False)
    outT = nc.declare_dram_parameter("outT", [H, BL], bf16, isOutput=True)
    memT = nc.declare_dram_parameter("memT", [H, BL], bf16, isOutput=True)
    if USE_FP8:
        # full-k fp8 weight copies; each gate loads only its first KF k-tiles
        wq8x = nc.declare_dram_parameter("wq8x", [NJS, 128, KT, 128], f8, isOutput=False)
        wq8h = nc.declare_dram_parameter("wq8h", [NJS, 128, KT, 128], f8, isOutput=False)
        wbx = nc.declare_dram_parameter("wbx", [NJS, 128, KB * 128], bf16, isOutput=False)
        wbh = nc.declare_dram_parameter("wbh", [NJS, 128, KB * 128], bf16, isOutput=False)
        xq8 = nc.declare_dram_parameter("xq8", [128, KT, BL], f8, isOutput=False)
        hq8 = nc.declare_dram_parameter("hq8", [128, KT, BL], f8, isOutput=False)
        scl = nc.declare_dram_parameter("scl", [128, 1], f32, isOutput=False)
    else:
        wbx = nc.declare_dram_parameter("wbx", [NJS, 128, E], bf16, isOutput=False)
        wbh = nc.declare_dram_parameter("wbh", [NJS, 128, H], bf16, isOutput=False)

    with tile.TileContext(nc) as tc:
        with (
            tc.tile_pool(name="resident", bufs=1) as resident,
            tc.tile_pool(name="wpool", bufs=4) as wpool,
            tc.tile_pool(name="cpool", bufs=2) as cpool,
            tc.tile_pool(name="psum", bufs=2, space="PSUM") as psum_pool,
            tc.tile_pool(name="gates", bufs=9) as gate_pool,
            tc.tile_pool(name="tmp", bufs=4) as tmp_pool,
            tc.tile_pool(name="outp", bufs=4) as out_pool,
        ):
            def split_dma(dst, src, nsplit, eng=None):
                eng = eng or nc.sync
                n = dst.shape[-1]
                per = n // nsplit
                for q in range(nsplit):
                    sl = slice(q * per, (q + 1) * per)
                    eng.dma_start(dst[:, sl], src[:, sl])

            bias_sb = resident.tile([128, NJI], f32, tag="bias")
            nc.sync.dma_start(bias_sb[:], bias[:])
            if USE_FP8:
                scl_sb = resident.tile([128, 1], f32, tag="scl")
                nc.sync.dma_start(scl_sb[:], scl[:])

            xt_k = [None] + [
                resident.tile([128, BL], bf16, tag=f"xt{k}", name=f"xt{k}")
                for k in range(1, KT)
            ]
            # k=0 is split into two half-tiles so the very first matmuls
            # (bc 0-1) depend on only 256KB of x instead of the full 512KB.
            xt0a = resident.tile([128, BL // 2], bf16, tag="xt0a", name="xt0a")
            xt0b = resident.tile([128, BL // 2], bf16, tag="xt0b", name="xt0b")

            def rhs_x(k, bc):
                if k == 0:
                    t = xt0a if bc < 2 else xt0b
                    return t[:, (bc % 2) * BC : (bc % 2 + 1) * BC]
                return xt_k[k][:, bc * BC : (bc + 1) * BC]
            # ht0/ht1 bf16 are never read when USE_FP8: every gate's h-side
            # k-tiles 0-1 come from hq8 (hwp is x-only), so skip them.
            HT0 = KF if USE_FP8 else 0
            ht_k = [None] * HT0 + [
                resident.tile([128, BL], bf16, tag=f"ht{k}", name=f"ht{k}")
                for k in range(HT0, KT)
            ]
            if USE_FP8:
                xq8_sb = resident.tile([128, KT, BL], f8, tag="xq8")
                hq8_sb = resident.tile([128, KT, BL], f8, tag="hq8")

            # Preloaded weight tiles: three hw_proj tiles (x-only — PE works
            # on these while h streams in) and the first i-gate tile.
            w_hwp0 = wpool.tile([128, E], bf16, tag="w")
            w_hwp1 = wpool.tile([128, E], bf16, tag="w")
            w_hwp2 = wpool.tile([128, E], bf16, tag="w")
            split_dma(xt0a, xT[0:128, : BL // 2], 2, eng=nc.gpsimd)
            split_dma(w_hwp0, whwp[0], 2, eng=nc.gpsimd)
            split_dma(xt0b, xT[0:128, BL // 2 :], 2, eng=nc.gpsimd)
            split_dma(xt_k[1], xT[128:256, :], 4, eng=nc.gpsimd)
            if USE_FP8:
                for q in range(4):
                    sl = slice(q * BL // 4, (q + 1) * BL // 4)
                    nc.gpsimd.dma_start(xq8_sb[:, :, sl], xq8[:, :, sl])
                for q in range(4):
                    sl = slice(q * BL // 4, (q + 1) * BL // 4)
                    nc.gpsimd.dma_start(hq8_sb[:, :, sl], hq8[:, :, sl])
            for k in range(2, 3):
                split_dma(xt_k[k], xT[k * 128 : (k + 1) * 128, :], 4)
            split_dma(w_hwp1, whwp[1], 2)
            split_dma(w_hwp2, whwp[2], 2)

            def load_gate_w(jt, eng=None):
                eng = eng or nc.sync
                if USE_FP8:
                    kf = gate_kf(jt)
                    kb = KT - kf
                    g8x = wpool.tile([128, kf, 128], f8, tag="w8x")
                    eng.dma_start(g8x[:], wq8x[jt][:, :kf, :])
                    gbx = gbh = None
                    if kb:
                        gbx = wpool.tile([128, kb * 128], bf16, tag="wbx",
                                         bufs=3)
                        eng.dma_start(gbx[:], wbx[jt][:, (KB - kb) * 128 :])
                    g8h = wpool.tile([128, kf, 128], f8, tag="w8h")
                    eng.dma_start(g8h[:], wq8h[jt][:, :kf, :])
                    if kb:
                        gbh = wpool.tile([128, kb * 128], bf16, tag="wbh",
                                         bufs=3)
                        eng.dma_start(gbh[:], wbh[jt][:, (KB - kb) * 128 :])
                    return (g8x, gbx, g8h, gbh)
                gbx = wpool.tile([128, E], bf16, tag="wbx")
                eng.dma_start(gbx[:], wbx[jt])
                gbh = wpool.tile([128, H], bf16, tag="wbh")
                eng.dma_start(gbh[:], wbh[jt])
                return (None, gbx, None, gbh)

            w_i0 = load_gate_w(0)
            for k in range(3, KT):
                split_dma(xt_k[k], xT[k * 128 : (k + 1) * 128, :], 4)
            # preload group-0 o/hw gate weights (first gates computed)
            w_o0 = load_gate_w(3 * NT)
            w_hw0 = load_gate_w(4 * NT)
            for k in range(HT0, KT):
                split_dma(ht_k[k], hT[k * 128 : (k + 1) * 128, :], 4)

            def gate_tile(jt, func, w=None, chunk_act=1):
                """Gate proj tile jt (0..NJS-1) -> activated gate (bf16)."""
                if w is None:
                    w = load_gate_w(jt)
                g8x, gbx, g8h, gbh = w
                kf = gate_kf(jt) if USE_FP8 else 0
                ps = psum_pool.tile([128, BL], f32, tag="ps")
                if USE_FP8:
                    for j in range(kf // 2):
                        for bc in range(NBC):
                            nc.tensor.matmul(
                                ps[:, bc * BC : (bc + 1) * BC],
                                g8x[:, 2 * j : 2 * j + 2, :],
                                xq8_sb[:, 2 * j : 2 * j + 2,
                                       bc * BC : (bc + 1) * BC],
                                start=(j == 0), stop=False, perf_mode=DR,
                            )
                for k in range(kf, KT):
                    lhsT = gbx[:, (k - kf) * 128 : (k - kf + 1) * 128]
                    for bc in range(NBC):
                        lo = bc * BC
                        nc.tensor.matmul(
                            ps[:, lo : lo + BC], lhsT, rhs_x(k, bc),
                            start=(not USE_FP8 and k == 0), stop=False,
                        )
                if USE_FP8:
                    for j in range(kf // 2):
                        for bc in range(NBC):
                            nc.tensor.matmul(
                                ps[:, bc * BC : (bc + 1) * BC],
                                g8h[:, 2 * j : 2 * j + 2, :],
                                hq8_sb[:, 2 * j : 2 * j + 2,
                                       bc * BC : (bc + 1) * BC],
                                start=False,
                                stop=(kf == KT and j == kf // 2 - 1),
                                perf_mode=DR,
                            )
                for k in range(kf, KT):
                    lhsT = gbh[:, (k - kf) * 128 : (k - kf + 1) * 128]
                    for bc in range(NBC):
                        lo = bc * BC
                        nc.tensor.matmul(
                            ps[:, lo : lo + BC], lhsT,
                            ht_k[k][:, bc * BC : (bc + 1) * BC],
                            start=False, stop=(k == KT - 1),
                        )
                g = gate_pool.tile([128, BL], bf16, tag="g")
                kw = {"scale": scl_sb[:, 0:1]} if USE_FP8 else {}
                cw = BL // chunk_act
                for a in range(chunk_act):
                    sl = slice(a * cw, (a + 1) * cw)
                    nc.scalar.activation(
                        g[:, sl], ps[:, sl], func,
                        bias=bias_sb[:, jt : jt + 1], **kw
                    )
                return g

            def hwp_tile(t, func=AF.Identity, w_i=None, chunk_act=1,
                         bc0=0, bc1=NBC):
                """hw_proj tile t (x-only, full bf16, unscaled)."""
                jt = 5 * NT + t
                if w_i is None:
                    w_i = wpool.tile([128, E], bf16, tag="w")
                    nc.sync.dma_start(w_i[:], whwp[t])
                width = (bc1 - bc0) * BC
                ps = psum_pool.tile([128, width], f32, tag="ps")
                for k in range(KT):
                    lhsT = w_i[:, k * 128 : (k + 1) * 128]
                    for bc in range(bc0, bc1):
                        lo = (bc - bc0) * BC
                        nc.tensor.matmul(
                            ps[:, lo : lo + BC], lhsT, rhs_x(k, bc),
                            start=(k == 0), stop=(k == KT - 1),
                        )
                g = gate_pool.tile([128, width], bf16, tag="g")
                cw = width // chunk_act
                for a in range(chunk_act):
                    sl = slice(a * cw, (a + 1) * cw)
                    nc.scalar.activation(
                        g[:, sl], ps[:, sl], func, bias=bias_sb[:, jt : jt + 1]
                    )
                return g

            mult, addop, subop = (
                mybir.AluOpType.mult,
                mybir.AluOpType.add,
                mybir.AluOpType.subtract,
            )

            # Head: hwp0/hwp1 k-interleaved across two PSUM tiles (each
            # landed x k-tile feeds 8 matmuls, halving the head's x-demand
            # rate), then hwp2 interleaved with o0 (fp8 stream — different
            # supply) the same way.
            ps_a = psum_pool.tile([128, BL], f32, tag="ps")
            ps_b = psum_pool.tile([128, BL], f32, tag="ps")
            for k in range(KT):
                la = w_hwp0[:, k * 128 : (k + 1) * 128]
                lb = w_hwp1[:, k * 128 : (k + 1) * 128]
                for bc in range(NBC):
                    lo = bc * BC
                    nc.tensor.matmul(ps_a[:, lo : lo + BC], la, rhs_x(k, bc),
                                     start=(k == 0), stop=(k == KT - 1))
                for bc in range(NBC):
                    lo = bc * BC
                    nc.tensor.matmul(ps_b[:, lo : lo + BC], lb, rhs_x(k, bc),
                                     start=(k == 0), stop=(k == KT - 1))
            hwp_pre = []
            for idx, ps in ((0, ps_a), (1, ps_b)):
                g = gate_pool.tile([128, BL], bf16, tag="g")
                jt = 5 * NT + idx
                for a in range(2):
                    sl = slice(a * BL // 2, (a + 1) * BL // 2)
                    nc.scalar.activation(
                        g[:, sl], ps[:, sl], AF.Identity,
                        bias=bias_sb[:, jt : jt + 1],
                    )
                hwp_pre.append(g)

            ps_c = psum_pool.tile([128, BL], f32, tag="ps")
            ps_d = psum_pool.tile([128, BL], f32, tag="ps")
            g8x_o, _, g8h_o, _ = w_o0
            for k in range(KT):
                lc = w_hwp2[:, k * 128 : (k + 1) * 128]
                for bc in range(NBC):
                    lo = bc * BC
                    nc.tensor.matmul(ps_c[:, lo : lo + BC], lc, rhs_x(k, bc),
                                     start=(k == 0), stop=(k == KT - 1))
                j = k if k < 4 else k - 4
                w8 = g8x_o if k < 4 else g8h_o
                aq = xq8_sb if k < 4 else hq8_sb
                for bc in range(NBC):
                    lo = bc * BC
                    nc.tensor.matmul(
                        ps_d[:, lo : lo + BC],
                        w8[:, 2 * j : 2 * j + 2, :],
                        aq[:, 2 * j : 2 * j + 2, lo : lo + BC],
                        start=(k == 0), stop=(k == KT - 1), perf_mode=DR,
                    )
            g = gate_pool.tile([128, BL], bf16, tag="g")
            for a in range(2):
                sl = slice(a * BL // 2, (a + 1) * BL // 2)
                nc.scalar.activation(
                    g[:, sl], ps_c[:, sl], AF.Identity,
                    bias=bias_sb[:, 5 * NT + 2 : 5 * NT + 3],
                )
            hwp_pre.append(g)
            o_g0 = gate_pool.tile([128, BL], bf16, tag="g")
            for a in range(2):
                sl = slice(a * BL // 2, (a + 1) * BL // 2)
                nc.scalar.activation(
                    o_g0[:, sl], ps_d[:, sl], AF.Sigmoid,
                    bias=bias_sb[:, 3 * NT : 3 * NT + 1],
                    scale=scl_sb[:, 0:1],
                )
            for t in range(NT):
                last = t == NT - 1
                if not last:
                    hwp = hwp_pre[t] if t < len(hwp_pre) else hwp_tile(t)
                if t == 0:
                    # group 0: o0 was computed in the head (interleaved with
                    # hwp2); hw0 (fp8-only) runs first here, extending the
                    # startup runway before xt4-7/ht2-7 must have landed
                    o_g = o_g0
                    hw_g = gate_tile(4 * NT + t, AF.Sigmoid, w=w_hw0)
                i_g = gate_tile(t, AF.Sigmoid, w=w_i0 if t == 0 else None)
                m_g = gate_tile(2 * NT + t, AF.Tanh)
                f_g = gate_tile(NT + t, AF.Sigmoid)

                ct = cpool.tile([128, BL], bf16, tag="c")
                nc.sync.dma_start(ct[:], cT[t * 128 : (t + 1) * 128, :])

                t1 = tmp_pool.tile([128, BL], bf16, tag="tmp")
                nc.vector.tensor_tensor(t1[:], i_g[:], m_g[:], mult)
                t2 = tmp_pool.tile([128, BL], bf16, tag="tmp")
                nc.vector.tensor_tensor(t2[:], f_g[:], ct[:], mult)
                mem = out_pool.tile([128, BL], bf16, tag="mem")
                nc.vector.tensor_tensor(mem[:], t1[:], t2[:], addop)
                nc.sync.dma_start(memT[t * 128 : (t + 1) * 128, :], mem[:])

                if t != 0:
                    o_g = gate_tile(3 * NT + t, AF.Sigmoid)

                tmem = tmp_pool.tile([128, BL], bf16, tag="tmp")
                nc.scalar.activation(tmem[:], mem[:], AF.Tanh)
                outp = tmp_pool.tile([128, BL], bf16, tag="tmp")
                nc.vector.tensor_tensor(outp[:], o_g[:], tmem[:], mult)

                if t != 0:
                    hw_g = gate_tile(4 * NT + t, AF.Sigmoid,
                                     chunk_act=4 if last else 1)

                if not last:
                    # out = hwp + hw*(outp - hwp), chunked so the blend
                    # pipelines with the output DMA.
                    u = tmp_pool.tile([128, BL], bf16, tag="tmp")
                    nc.vector.tensor_tensor(u[:], outp[:], hwp[:], subop)
                    for e in range(2):
                        sl = slice(e * (BL // 2), (e + 1) * (BL // 2))
                        v = tmp_pool.tile([128, BL // 2], bf16, tag="v")
                        nc.vector.tensor_tensor(v[:], hw_g[:, sl], u[:, sl], mult)
                        outf = out_pool.tile([128, BL // 2], bf16, tag="out")
                        nc.vector.tensor_tensor(outf[:], v[:], hwp[:, sl], addop)
                        eng = nc.sync if e == 0 else nc.scalar
                        eng.dma_start(outT[t * 128 : (t + 1) * 128, sl], outf[:])
                else:
                    # Last group: hwp computed LAST in per-bc chunks with
                    # fused blend+DMA; final 512 cols split into two 256-col
                    # chunks so the post-matmul tail is minimal.
                    w_hwp_l = wpool.tile([128, E], bf16, tag="w")
                    nc.sync.dma_start(w_hwp_l[:], whwp[t])
                    jt = 5 * NT + t
                    chunks = [(0, BC), (BC, BC), (2 * BC, BC),
                              (3 * BC, BC // 2), (3 * BC + BC // 2, BC // 2)]
                    for ci, (lo, cw) in enumerate(chunks):
                        sl = slice(lo, lo + cw)
                        ps = psum_pool.tile([128, cw], f32, tag="ps")
                        for k in range(KT):
                            nc.tensor.matmul(
                                ps[:],
                                w_hwp_l[:, k * 128 : (k + 1) * 128],
                                rhs_x(k, lo // BC)[:, lo % BC : lo % BC + cw],
                                start=(k == 0),
                                stop=(k == KT - 1),
                            )
                        hwp_c = tmp_pool.tile([128, cw], bf16, tag="v")
                        nc.scalar.activation(
                            hwp_c[:], ps[:], AF.Identity,
                            bias=bias_sb[:, jt : jt + 1],
                        )
                        # alternate blend lanes vector/gpsimd so the final
                        # chunks' element-wise chains run in parallel
                        ve = nc.vector if ci % 2 == 0 else nc.gpsimd
                        u = tmp_pool.tile([128, cw], bf16, tag="v")
                        ve.tensor_tensor(u[:], outp[:, sl], hwp_c[:], subop)
                        v = tmp_pool.tile([128, cw], bf16, tag="v")
                        ve.tensor_tensor(v[:], hw_g[:, sl], u[:], mult)
                        outf = out_pool.tile([128, cw], bf16, tag="out")
                        ve.tensor_tensor(outf[:], v[:], hwp_c[:], addop)
                        eng = nc.sync if ci % 2 == 0 else nc.scalar
                        eng.dma_start(outT[t * 128 : (t + 1) * 128, sl], outf[:])

    nc.compile()
    return nc


_NC_CACHE = None


def _get_nc():
    global _NC_CACHE
    if _NC_CACHE is None:
        _NC_CACHE = build_nc()
    return _NC_CACHE


def _pack_weights(W, njt):
    # W [njt*128 j, K e] -> [njt, 128 p, K] with [jt, p, k*128+m] = W[jt*128+m, k*128+p]
    W = np.asarray(W, np.float32)
    K = W.shape[1]
    kt = K // 128
    return np.ascontiguousarray(
        W.reshape(njt, 128, kt, 128).transpose(0, 3, 2, 1).reshape(njt, 128, K)
    )


def _q8(a, sc):
    return np.clip(np.asarray(a, np.float32) * sc, -240.0, 240.0).astype(F8E4)


def prepare_in_maps(x, h, c, Wi, bi, Ws, bs):
    x = np.asarray(x, np.float32)
    h = np.asarray(h, np.float32)
    Wi = np.asarray(Wi, np.float32)
    Ws = np.asarray(Ws, np.float32)
    Wg = Wi[: 5 * H]

    bias_comb = np.concatenate(
        [np.asarray(bi[: 5 * H], np.float32) + np.asarray(bs, np.float32),
         np.asarray(bi[5 * H :], np.float32)]
    )
    bias_pack = np.ascontiguousarray(bias_comb.reshape(NJI, 128).T).astype(np.float32)
    whwp_p = _pack_weights(Wi[5 * H :], NT).astype(BF16)

    common = {"bias": bias_pack, "whwp": whwp_p}
    kf = KF * 128
    if USE_FP8:
        sW = np.float32(224.0 / max(np.abs(Wg).max(), np.abs(Ws).max()))
        sA = np.float32(224.0 / max(np.abs(x).max(), np.abs(h).max()))
        S = np.float32(sW * sA)
        # [jt, m, j, p] -> [jt, p, j, m]
        wq8x_p = _q8(Wg.reshape(NJS, 128, KT, 128).transpose(0, 3, 2, 1), sW)
        wq8h_p = _q8(Ws.reshape(NJS, 128, KT, 128).transpose(0, 3, 2, 1), sW)
        wbx_p = np.ascontiguousarray(
            _pack_weights(Wg * S, NJS)[:, :, kf:]).astype(BF16)
        wbh_p = np.ascontiguousarray(
            _pack_weights(Ws * S, NJS)[:, :, kf:]).astype(BF16)
        common.update(
            wq8x=np.ascontiguousarray(wq8x_p),
            wq8h=np.ascontiguousarray(wq8h_p),
            wbx=wbx_p,
            wbh=wbh_p,
            scl=np.full((128, 1), 1.0 / S, np.float32),
        )
    else:
        common.update(
            wbx=_pack_weights(Wg, NJS).astype(BF16),
            wbh=_pack_weights(Ws, NJS).astype(BF16),
        )

    in_maps = []
    for i in range(N_CORES):
        s = slice(i * BL, (i + 1) * BL)
        m = {
            "xT": np.ascontiguousarray(x[s].T).astype(BF16),
            "hT": np.ascontiguousarray(h[s].T).astype(BF16),
            "cT": np.ascontiguousarray(np.asarray(c[s], np.float32).T).astype(BF16),
        }
        if USE_FP8:
            # [p, j, b] with value in[b, j*128+p]
            m["xq8"] = np.ascontiguousarray(
                _q8(x[s].T.reshape(KT, 128, BL).transpose(1, 0, 2), sA))
            m["hq8"] = np.ascontiguousarray(
                _q8(h[s].T.reshape(KT, 128, BL).transpose(1, 0, 2), sA))
        m.update(common)
        in_maps.append(m)
    return in_maps


def run(in_maps, trace=False):
    nc = _get_nc()
    res = run_bass_kernel_spmd(nc, in_maps, core_ids=list(range(N_CORES)), trace=trace)
    out = np.empty((B, H), np.float32)
    mem = np.empty((B, H), np.float32)
    for i in range(N_CORES):
        s = slice(i * BL, (i + 1) * BL)
        out[s] = res.results[i]["outT"].astype(np.float32).T
        mem[s] = res.results[i]["memT"].astype(np.float32).T
    return (out, mem), res


def kernel(x, h, c, Wi, bi, Ws, bs):
    in_maps = prepare_in_maps(x, h, c, Wi, bi, Ws, bs)
    (out, mem), _ = run(in_maps, trace=False)
    return out, mem


# revision 49
# speedup vs baseline: 1.0210x; 1.0210x over previous
"""AugmentedLSTMCell on 8 TRN2 NeuronCores — data-parallel over batch.

Layout: feature-on-partition (transposed). Per core: B_loc=2048 batch rows.
  proj.T[j, b] = sum_e W[j, e] * in[b, e]
  lhsT tiles  = W.T blocks [128e, 128j]  (host pre-packed, bf16)
  rhs         = in.T        [128e, 2048b] (host pre-transposed, bf16)
  psum [128j, 2048b] accumulates the Wi-proj and Ws-proj contraction
  (the "fused = proj_in + proj_st" add comes free via PSUM accumulation).
  ScalarE applies per-feature bias + sigmoid/tanh straight out of PSUM.
Host transposes outputs back to [B, H].

Perf structure:
  - partial fp8: the first 2 of 8 contraction k-tiles of every GATE
    projection run as one DoubleRow fp8 matmul (2 k-tiles per
    instruction at ~1.4-1.8x bf16 throughput). hwp (the highway
    projection, which enters the output linearly) stays full bf16, as
    do k-tiles 2-7 of the gates, so the added quantization error is
    ~sqrt(0.25) of full-fp8 -> rel_err ~1.3e-2 (limit 2e-2).
    fp8 product scale S = sW*sA is folded out via the activation's
    scale operand; the bf16 gate weights are pre-scaled by S on host.
  - outputs written as bf16 (halves output HBM traffic; host upcasts)
  - last tile group reordered: hwp computed LAST in per-bc chunks with
    fused blend+DMA so the post-matmul tail is ~3us instead of ~7us
"""
import sys
import types

sys.path.insert(0, "/opt/trn_rl_repo")
sys.path.insert(0, "/root/.axon_site")

# Shim antenv.axon_hooks (missing on this image) so trace=True can profile.
if "antenv.axon_hooks" not in sys.modules:
    _hooks = types.ModuleType("antenv.axon_hooks")
    _state = {"hook": None}
    _hooks.set_axon_ntff_profile_hook = lambda h: _state.__setitem__("hook", h)
    _hooks.get_axon_ntff_profile_hook = lambda: _state["hook"]
    sys.modules["antenv.axon_hooks"] = _hooks
    try:
        from trn_agent_boot.trn_boot import _ntff_profile_via_ctypes

        _hooks.set_axon_ntff_profile_hook(
            _ntff_profile_via_ctypes("/opt/axon/libaxon_pjrt.so")
        )
    except Exception:
        pass

import numpy as np
import ml_dtypes

import concourse.bass as bass
import concourse.bacc as bacc
import concourse.mybir as mybir
from concourse import tile
from concourse.bass_utils import run_bass_kernel_spmd

BF16 = ml_dtypes.bfloat16
F8E4 = ml_dtypes.float8_e4m3fn

N_CORES = 8
B, E, H = 16384, 1024, 1024
BL = B // N_CORES          # 2048 batch rows per core
KT = E // 128              # 8 contraction k-tiles
KF = 2                     # minimum fp8 DoubleRow k-tiles (the m gate)
KF4 = 4                    # fp8 k-tiles for i/f gates


def gate_kf(jt):
    """fp8 k-tiles per gate tile: m->2 (dominates mem error), i/f->6,
    o/hw->8 (their error reaches `out` only via sigmoid-compressed paths)."""
    if 2 * NT <= jt < 3 * NT:
        return 2          # m
    if jt >= 3 * NT:
        return 8          # o, hw
    return 6              # i, f
NJI = 6 * H // 128         # 48 feature tiles of proj_in
NJS = 5 * H // 128         # 40 feature tiles of proj_st (the gates)
NT = H // 128              # 8 H-slices
BC = 512                   # matmul moving free dim (one PSUM bank)
NBC = BL // BC             # batch chunks per matmul group
KB = KT - KF               # bf16 k-tiles per gate side

USE_FP8 = True

AF = mybir.ActivationFunctionType
DR = mybir.MatmulPerfMode.DoubleRow


def build_nc():
    nc = bacc.Bacc(None, target_bir_lowering=False)
    f32, bf16 = mybir.dt.float32, mybir.dt.bfloat16
    f8 = mybir.dt.float8e4

    xT = nc.declare_dram_parameter("xT", [E, BL], bf16, isOutput=False)
    hT = nc.declare_dram_parameter("hT", [H, BL], bf16, isOutput=False)
    cT = nc.declare_dram_parameter("cT", [H, BL], bf16, isOutput=False)
    whwp = nc.declare_dram_parameter("whwp", [NT, 128, E], bf16, isOutput=False)
    bias = nc.declare_dram_parameter("bias", [128, NJI], f32, isOutput=False)
    outT = nc.declare_dram_parameter("outT", [H, BL], bf16, isOutput=True)
    memT = nc.declare_dram_parameter("memT", [H, BL], bf16, isOutput=True)
    if USE_FP8:
        # full-k fp8 weight copies; each gate loads only its first KF k-tiles
        wq8x = nc.declare_dram_parameter("wq8x", [NJS, 128, KT, 128], f8, isOutput=False)
        wq8h = nc.declare_dram_parameter("wq8h", [NJS, 128, KT, 128], f8, isOutput=False)
        wbx = nc.declare_dram_parameter("wbx", [NJS, 128, KB * 128], bf16, isOutput=False)
        wbh = nc.declare_dram_parameter("wbh", [NJS, 128, KB * 128], bf16, isOutput=False)
        xq8 = nc.declare_dram_parameter("xq8", [128, KT, BL], f8, isOutput=False)
        hq8 = nc.declare_dram_parameter("hq8", [128, KT, BL], f8, isOutput=False)
        scl = nc.declare_dram_parameter("scl", [128, 1], f32, isOutput=False)
    else:
        wbx = nc.declare_dram_parameter("wbx", [NJS, 128, E], bf16, isOutput=False)
        wbh = nc.declare_dram_parameter("wbh", [NJS, 128, H], bf16, isOutput=False)

    with tile.TileContext(nc) as tc:
        with (
            tc.tile_pool(name="resident", bufs=1) as resident,
            tc.tile_pool(name="wpool", bufs=4) as wpool,
            tc.tile_pool(name="cpool", bufs=2) as cpool,
            tc.tile_pool(name="psum", bufs=2, space="PSUM") as psum_pool,
            tc.tile_pool(name="gates", bufs=9) as gate_pool,
            tc.tile_pool(name="tmp", bufs=4) as tmp_pool,
            tc.tile_pool(name="outp", bufs=4) as out_pool,
        ):
            def split_dma(dst, src, nsplit, eng=None):
                eng = eng or nc.sync
                n = dst.shape[-1]
                per = n // nsplit
                for q in range(nsplit):
                    sl = slice(q * per, (q + 1) * per)
                    eng.dma_start(dst[:, sl], src[:, sl])

            bias_sb = resident.tile([128, NJI], f32, tag="bias")
            nc.sync.dma_start(bias_sb[:], bias[:])
            if USE_FP8:
                scl_sb = resident.tile([128, 1], f32, tag="scl")
                nc.sync.dma_start(scl_sb[:], scl[:])

            xt_k = [None] + [
                resident.tile([128, BL], bf16, tag=f"xt{k}", name=f"xt{k}")
                for k in range(1, KT)
            ]
            # k=0 is split into two half-tiles so the very first matmuls
            # (bc 0-1) depend on only 256KB of x instead of the full 512KB.
            xt0a = resident.tile([128, BL // 2], bf16, tag="xt0a", name="xt0a")
            xt0b = resident.tile([128, BL // 2], bf16, tag="xt0b", name="xt0b")

            def rhs_x(k, bc):
                if k == 0:
                    t = xt0a if bc < 2 else xt0b
                    return t[:, (bc % 2) * BC : (bc % 2 + 1) * BC]
                return xt_k[k][:, bc * BC : (bc + 1) * BC]
            # ht0/ht1 bf16 are never read when USE_FP8: every gate's h-side
            # k-tiles 0-1 come from hq8 (hwp is x-only), so skip them.
            HT0 = KF if USE_FP8 else 0
            ht_k = [None] * HT0 + [
                resident.tile([128, BL], bf16, tag=f"ht{k}", name=f"ht{k}")
                for k in range(HT0, KT)
            ]
            if USE_FP8:
                xq8_sb = resident.tile([128, KT, BL], f8, tag="xq8")
                hq8_sb = resident.tile([128, KT, BL], f8, tag="hq8")

            # Preloaded weight tiles: three hw_proj tiles (x-only — PE works
            # on these while h streams in) and the first i-gate tile.
            w_hwp0 = wpool.tile([128, E], bf16, tag="w")
            w_hwp1 = wpool.tile([128, E], bf16, tag="w")
            w_hwp2 = wpool.tile([128, E], bf16, tag="w")
            split_dma(xt0a, xT[0:128, : BL // 2], 2, eng=nc.gpsimd)
            split_dma(w_hwp0, whwp[0], 2, eng=nc.gpsimd)
            split_dma(xt0b, xT[0:128, BL // 2 :], 2, eng=nc.gpsimd)
            split_dma(xt_k[1], xT[128:256, :], 4, eng=nc.gpsimd)
            if USE_FP8:
                for q in range(4):
                    sl = slice(q * BL // 4, (q + 1) * BL // 4)
                    nc.gpsimd.dma_start(xq8_sb[:, :, sl], xq8[:, :, sl])
                for q in range(4):
                    sl = slice(q * BL // 4, (q + 1) * BL // 4)
                    nc.gpsimd.dma_start(hq8_sb[:, :, sl], hq8[:, :, sl])
            for k in range(2, 3):
                split_dma(xt_k[k], xT[k * 128 : (k + 1) * 128, :], 4)
            split_dma(w_hwp1, whwp[1], 2)
            split_dma(w_hwp2, whwp[2], 2)

            def load_gate_w(jt, eng=None):
                eng = eng or nc.sync
                if USE_FP8:
                    kf = gate_kf(jt)
                    kb = KT - kf
                    g8x = wpool.tile([128, kf, 128], f8, tag="w8x")
                    eng.dma_start(g8x[:], wq8x[jt][:, :kf, :])
                    gbx = gbh = None
                    if kb:
                        gbx = wpool.tile([128, kb * 128], bf16, tag="wbx",
                                         bufs=3)
                        eng.dma_start(gbx[:], wbx[jt][:, (KB - kb) * 128 :])
                    g8h = wpool.tile([128, kf, 128], f8, tag="w8h")
                    eng.dma_start(g8h[:], wq8h[jt][:, :kf, :])
                    if kb:
                        gbh = wpool.tile([128, kb * 128], bf16, tag="wbh",
                                         bufs=3)
                        eng.dma_start(gbh[:], wbh[jt][:, (KB - kb) * 128 :])
                    return (g8x, gbx, g8h, gbh)
                gbx = wpool.tile([128, E], bf16, tag="wbx")
                eng.dma_start(gbx[:], wbx[jt])
                gbh = wpool.tile([128, H], bf16, tag="wbh")
                eng.dma_start(gbh[:], wbh[jt])
                return (None, gbx, None, gbh)

            w_i0 = load_gate_w(0)
            for k in range(3, KT):
                split_dma(xt_k[k], xT[k * 128 : (k + 1) * 128, :], 4)
            # preload group-0 o/hw gate weights (first gates computed)
            w_o0 = load_gate_w(3 * NT)
            w_hw0 = load_gate_w(4 * NT)
            for k in range(HT0, KT):
                split_dma(ht_k[k], hT[k * 128 : (k + 1) * 128, :], 4)

            def gate_tile(jt, func, w=None, chunk_act=1):
                """Gate proj tile jt (0..NJS-1) -> activated gate (bf16)."""
                if w is None:
                    w = load_gate_w(jt)
                g8x, gbx, g8h, gbh = w
                kf = gate_kf(jt) if USE_FP8 else 0
                ps = psum_pool.tile([128, BL], f32, tag="ps")
                if USE_FP8:
                    for j in range(kf // 2):
                        for bc in range(NBC):
                            nc.tensor.matmul(
                                ps[:, bc * BC : (bc + 1) * BC],
                                g8x[:, 2 * j : 2 * j + 2, :],
                                xq8_sb[:, 2 * j : 2 * j + 2,
                                       bc * BC : (bc + 1) * BC],
                                start=(j == 0), stop=False, perf_mode=DR,
                            )
                for k in range(kf, KT):
                    lhsT = gbx[:, (k - kf) * 128 : (k - kf + 1) * 128]
                    for bc in range(NBC):
                        lo = bc * BC
                        nc.tensor.matmul(
                            ps[:, lo : lo + BC], lhsT, rhs_x(k, bc),
                            start=(not USE_FP8 and k == 0), stop=False,
                        )
                if USE_FP8:
                    for j in range(kf // 2):
                        for bc in range(NBC):
                            nc.tensor.matmul(
                                ps[:, bc * BC : (bc + 1) * BC],
                                g8h[:, 2 * j : 2 * j + 2, :],
                                hq8_sb[:, 2 * j : 2 * j + 2,
                                       bc * BC : (bc + 1) * BC],
                                start=False,
                                stop=(kf == KT and j == kf // 2 - 1),
                                perf_mode=DR,
                            )
                for k in range(kf, KT):
                    lhsT = gbh[:, (k - kf) * 128 : (k - kf + 1) * 128]
                    for bc in range(NBC):
                        lo = bc * BC
                        nc.tensor.matmul(
                            ps[:, lo : lo + BC], lhsT,
                            ht_k[k][:, bc * BC : (bc + 1) * BC],
                            start=False, stop=(k == KT - 1),
                        )
                g = gate_pool.tile([128, BL], bf16, tag="g")
                kw = {"scale": scl_sb[:, 0:1]} if USE_FP8 else {}
                cw = BL // chunk_act
                for a in range(chunk_act):
                    sl = slice(a * cw, (a + 1) * cw)
                    nc.scalar.activation(
                        g[:, sl], ps[:, sl], func,
                        bias=bias_sb[:, jt : jt + 1], **kw
                    )
                return g

            def hwp_tile(t, func=AF.Identity, w_i=None, chunk_act=1,
                         bc0=0, bc1=NBC):
                """hw_proj tile t (x-only, full bf16, unscaled)."""
                jt = 5 * NT + t
                if w_i is None:
                    w_i = wpool.tile([128, E], bf16, tag="w")
                    nc.sync.dma_start(w_i[:], whwp[t])
                width = (bc1 - bc0) * BC
                ps = psum_pool.tile([128, width], f32, tag="ps")
                for k in range(KT):
                    lhsT = w_i[:, k * 128 : (k + 1) * 128]
                    for bc in range(bc0, bc1):
                        lo = (bc - bc0) * BC
                        nc.tensor.matmul(
                            ps[:, lo : lo + BC], lhsT, rhs_x(k, bc),
                            start=(k == 0), stop=(k == KT - 1),
                        )
                g = gate_pool.tile([128, width], bf16, tag="g")
                cw = width // chunk_act
                for a in range(chunk_act):
                    sl = slice(a * cw, (a + 1) * cw)
                    nc.scalar.activation(
                        g[:, sl], ps[:, sl], func, bias=bias_sb[:, jt : jt + 1]
                    )
                return g

            mult, addop, subop = (
                mybir.AluOpType.mult,
                mybir.AluOpType.add,
                mybir.AluOpType.subtract,
            )

            hwp_pre = [
                hwp_tile(0, w_i=w_hwp0),
                hwp_tile(1, w_i=w_hwp1),
                hwp_tile(2, w_i=w_hwp2),
            ]
            for t in range(NT):
                last = t == NT - 1
                if not last:
                    hwp = hwp_pre[t] if t < len(hwp_pre) else hwp_tile(t)
                if t == 0:
                    # group 0: the fp8-only o/hw gates first — they need no
                    # bf16 x/h k-tiles, extending the startup runway before
                    # xt4-7/ht2-7 must have landed
                    o_g = gate_tile(3 * NT + t, AF.Sigmoid, w=w_o0)
                    hw_g = gate_tile(4 * NT + t, AF.Sigmoid, w=w_hw0)
                i_g = gate_tile(t, AF.Sigmoid, w=w_i0 if t == 0 else None)
                m_g = gate_tile(2 * NT + t, AF.Tanh)
                f_g = gate_tile(NT + t, AF.Sigmoid)

                ct = cpool.tile([128, BL], bf16, tag="c")
                nc.sync.dma_start(ct[:], cT[t * 128 : (t + 1) * 128, :])

                t1 = tmp_pool.tile([128, BL], bf16, tag="tmp")
                nc.vector.tensor_tensor(t1[:], i_g[:], m_g[:], mult)
                t2 = tmp_pool.tile([128, BL], bf16, tag="tmp")
                nc.vector.tensor_tensor(t2[:], f_g[:], ct[:], mult)
                mem = out_pool.tile([128, BL], bf16, tag="mem")
                nc.vector.tensor_tensor(mem[:], t1[:], t2[:], addop)
                nc.sync.dma_start(memT[t * 128 : (t + 1) * 128, :], mem[:])

                if t != 0:
                    o_g = gate_tile(3 * NT + t, AF.Sigmoid)

                tmem = tmp_pool.tile([128, BL], bf16, tag="tmp")
                nc.scalar.activation(tmem[:], mem[:], AF.Tanh)
                outp = tmp_pool.tile([128, BL], bf16, tag="tmp")
                nc.vector.tensor_tensor(outp[:], o_g[:], tmem[:], mult)

                if t != 0:
                    hw_g = gate_tile(4 * NT + t, AF.Sigmoid,
                                     chunk_act=4 if last else 1)

                if not last:
                    # out = hwp + hw*(outp - hwp), chunked so the blend
                    # pipelines with the output DMA.
                    u = tmp_pool.tile([128, BL], bf16, tag="tmp")
                    nc.vector.tensor_tensor(u[:], outp[:], hwp[:], subop)
                    for e in range(2):
                        sl = slice(e * (BL // 2), (e + 1) * (BL // 2))
                        v = tmp_pool.tile([128, BL // 2], bf16, tag="v")
                        nc.vector.tensor_tensor(v[:], hw_g[:, sl], u[:, sl], mult)
                        outf = out_pool.tile([128, BL // 2], bf16, tag="out")
                        nc.vector.tensor_tensor(outf[:], v[:], hwp[:, sl], addop)
                        eng = nc.sync if e == 0 else nc.scalar
                        eng.dma_start(outT[t * 128 : (t + 1) * 128, sl], outf[:])
                else:
                    # Last group: hwp computed LAST in per-bc chunks with
                    # fused blend+DMA; final 512 cols split into two 256-col
                    # chunks so the post-matmul tail is minimal.
                    w_hwp_l = wpool.tile([128, E], bf16, tag="w")
                    nc.sync.dma_start(w_hwp_l[:], whwp[t])
                    jt = 5 * NT + t
                    chunks = [(0, BC), (BC, BC), (2 * BC, BC),
                              (3 * BC, BC // 2), (3 * BC + BC // 2, BC // 2)]
                    for ci, (lo, cw) in enumerate(chunks):
                        sl = slice(lo, lo + cw)
                        ps = psum_pool.tile([128, cw], f32, tag="ps")
                        for k in range(KT):
                            nc.tensor.matmul(
                                ps[:],
                                w_hwp_l[:, k * 128 : (k + 1) * 128],
                                rhs_x(k, lo // BC)[:, lo % BC : lo % BC + cw],
                                start=(k == 0),
                                stop=(k == KT - 1),
                            )
                        hwp_c = tmp_pool.tile([128, cw], bf16, tag="v")
                        nc.scalar.activation(
                            hwp_c[:], ps[:], AF.Identity,
                            bias=bias_sb[:, jt : jt + 1],
                        )
                        # alternate blend lanes vector/gpsimd so the final
                        # chunks' element-wise chains run in parallel
                        ve = nc.vector if ci % 2 == 0 else nc.gpsimd
                        u = tmp_pool.tile([128, cw], bf16, tag="v")
                        ve.tensor_tensor(u[:], outp[:, sl], hwp_c[:], subop)
                        v = tmp_pool.tile([128, cw], bf16, tag="v")
                        ve.tensor_tensor(v[:], hw_g[:, sl], u[:], mult)
                        outf = out_pool.tile([128, cw], bf16, tag="out")
                        ve.tensor_tensor(outf[:], v[:], hwp_c[:], addop)
                        eng = nc.sync if ci % 2 == 0 else nc.scalar
                        eng.dma_start(outT[t * 128 : (t + 1) * 128, sl], outf[:])

    nc.compile()
    return nc


_NC_CACHE = None


def _get_nc():
    global _NC_CACHE
    if _NC_CACHE is None:
        _NC_CACHE = build_nc()
    return _NC_CACHE


def _pack_weights(W, njt):
    # W [njt*128 j, K e] -> [njt, 128 p, K] with [jt, p, k*128+m] = W[jt*128+m, k*128+p]
    W = np.asarray(W, np.float32)
    K = W.shape[1]
    kt = K // 128
    return np.ascontiguousarray(
        W.reshape(njt, 128, kt, 128).transpose(0, 3, 2, 1).reshape(njt, 128, K)
    )


def _q8(a, sc):
    return np.clip(np.asarray(a, np.float32) * sc, -240.0, 240.0).astype(F8E4)


def prepare_in_maps(x, h, c, Wi, bi, Ws, bs):
    x = np.asarray(x, np.float32)
    h = np.asarray(h, np.float32)
    Wi = np.asarray(Wi, np.float32)
    Ws = np.asarray(Ws, np.float32)
    Wg = Wi[: 5 * H]

    bias_comb = np.concatenate(
        [np.asarray(bi[: 5 * H], np.float32) + np.asarray(bs, np.float32),
         np.asarray(bi[5 * H :], np.float32)]
    )
    bias_pack = np.ascontiguousarray(bias_comb.reshape(NJI, 128).T).astype(np.float32)
    whwp_p = _pack_weights(Wi[5 * H :], NT).astype(BF16)

    common = {"bias": bias_pack, "whwp": whwp_p}
    kf = KF * 128
    if USE_FP8:
        sW = np.float32(224.0 / max(np.abs(Wg).max(), np.abs(Ws).max()))
        sA = np.float32(224.0 / max(np.abs(x).max(), np.abs(h).max()))
        S = np.float32(sW * sA)
        # [jt, m, j, p] -> [jt, p, j, m]
        wq8x_p = _q8(Wg.reshape(NJS, 128, KT, 128).transpose(0, 3, 2, 1), sW)
        wq8h_p = _q8(Ws.reshape(NJS, 128, KT, 128).transpose(0, 3, 2, 1), sW)
        wbx_p = np.ascontiguousarray(
            _pack_weights(Wg * S, NJS)[:, :, kf:]).astype(BF16)
        wbh_p = np.ascontiguousarray(
            _pack_weights(Ws * S, NJS)[:, :, kf:]).astype(BF16)
        common.update(
            wq8x=np.ascontiguousarray(wq8x_p),
            wq8h=np.ascontiguousarray(wq8h_p),
            wbx=wbx_p,
            wbh=wbh_p,
            scl=np.full((128, 1), 1.0 / S, np.float32),
        )
    else:
        common.update(
            wbx=_pack_weights(Wg, NJS).astype(BF16),
            wbh=_pack_weights(Ws, NJS).astype(BF16),
        )

    in_maps = []
    for i in range(N_CORES):
        s = slice(i * BL, (i + 1) * BL)
        m = {
            "xT": np.ascontiguousarray(x[s].T).astype(BF16),
            "hT": np.ascontiguousarray(h[s].T).astype(BF16),
            "cT": np.ascontiguousarray(np.asarray(c[s], np.float32).T).astype(BF16),
        }
        if USE_FP8:
            # [p, j, b] with value in[b, j*128+p]
            m["xq8"] = np.ascontiguousarray(
                _q8(x[s].T.reshape(KT, 128, BL).transpose(1, 0, 2), sA))
            m["hq8"] = np.ascontiguousarray(
                _q8(h[s].T.reshape(KT, 128, BL).transpose(1, 0, 2), sA))
        m.update(common)
        in_maps.append(m)
    return in_maps


def run(in_maps, trace=False):
    nc = _get_nc()
    res = run_bass_kernel_spmd(nc, in_maps, core_ids=list(range(N_CORES)), trace=trace)
    out = np.empty((B, H), np.float32)
    mem = np.empty((B, H), np.float32)
    for i in range(N_CORES):
        s = slice(i * BL, (i + 1) * BL)
        out[s] = res.results[i]["outT"].astype(np.float32).T
        mem[s] = res.results[i]["memT"].astype(np.float32).T
    return (out, mem), res


def kernel(x, h, c, Wi, bi, Ws, bs):
    in_maps = prepare_in_maps(x, h, c, Wi, bi, Ws, bs)
    (out, mem), _ = run(in_maps, trace=False)
    return out, mem


# revision 50
# speedup vs baseline: 1.0848x; 1.0625x over previous
"""AugmentedLSTMCell on 8 TRN2 NeuronCores — data-parallel over batch.

Layout: feature-on-partition (transposed). Per core: B_loc=2048 batch rows.
  proj.T[j, b] = sum_e W[j, e] * in[b, e]
  lhsT tiles  = W.T blocks [128e, 128j]  (host pre-packed, bf16)
  rhs         = in.T        [128e, 2048b] (host pre-transposed, bf16)
  psum [128j, 2048b] accumulates the Wi-proj and Ws-proj contraction
  (the "fused = proj_in + proj_st" add comes free via PSUM accumulation).
  ScalarE applies per-feature bias + sigmoid/tanh straight out of PSUM.
Host transposes outputs back to [B, H].

Perf structure:
  - partial fp8: the first 2 of 8 contraction k-tiles of every GATE
    projection run as one DoubleRow fp8 matmul (2 k-tiles per
    instruction at ~1.4-1.8x bf16 throughput). hwp (the highway
    projection, which enters the output linearly) stays full bf16, as
    do k-tiles 2-7 of the gates, so the added quantization error is
    ~sqrt(0.25) of full-fp8 -> rel_err ~1.3e-2 (limit 2e-2).
    fp8 product scale S = sW*sA is folded out via the activation's
    scale operand; the bf16 gate weights are pre-scaled by S on host.
  - outputs written as bf16 (halves output HBM traffic; host upcasts)
  - last tile group reordered: hwp computed LAST in per-bc chunks with
    fused blend+DMA so the post-matmul tail is ~3us instead of ~7us
"""
import sys
import types

sys.path.insert(0, "/opt/trn_rl_repo")
sys.path.insert(0, "/root/.axon_site")

# Shim antenv.axon_hooks (missing on this image) so trace=True can profile.
if "antenv.axon_hooks" not in sys.modules:
    _hooks = types.ModuleType("antenv.axon_hooks")
    _state = {"hook": None}
    _hooks.set_axon_ntff_profile_hook = lambda h: _state.__setitem__("hook", h)
    _hooks.get_axon_ntff_profile_hook = lambda: _state["hook"]
    sys.modules["antenv.axon_hooks"] = _hooks
    try:
        from trn_agent_boot.trn_boot import _ntff_profile_via_ctypes

        _hooks.set_axon_ntff_profile_hook(
            _ntff_profile_via_ctypes("/opt/axon/libaxon_pjrt.so")
        )
    except Exception:
        pass

import numpy as np
import ml_dtypes

import concourse.bass as bass
import concourse.bacc as bacc
import concourse.mybir as mybir
from concourse import tile
from concourse.bass_utils import run_bass_kernel_spmd

BF16 = ml_dtypes.bfloat16
F8E4 = ml_dtypes.float8_e4m3fn

N_CORES = 8
B, E, H = 16384, 1024, 1024
BL = B // N_CORES          # 2048 batch rows per core
KT = E // 128              # 8 contraction k-tiles
KF = 2                     # minimum fp8 DoubleRow k-tiles (the m gate)
KF4 = 4                    # fp8 k-tiles for i/f gates


def gate_kf(jt):
    """fp8 k-tiles per gate tile: m->2 (it dominates the mem error budget);
    i/f/o/hw run fully fp8 (their error is damped by sigmoid slopes /
    bounded tanh before reaching the outputs)."""
    if 2 * NT <= jt < 3 * NT:
        return 2          # m
    return 8              # i, f, o, hw
NJI = 6 * H // 128         # 48 feature tiles of proj_in
NJS = 5 * H // 128         # 40 feature tiles of proj_st (the gates)
NT = H // 128              # 8 H-slices
BC = 512                   # matmul moving free dim (one PSUM bank)
NBC = BL // BC             # batch chunks per matmul group
KB = KT - KF               # bf16 k-tiles per gate side

USE_FP8 = True

AF = mybir.ActivationFunctionType
DR = mybir.MatmulPerfMode.DoubleRow


def build_nc():
    nc = bacc.Bacc(None, target_bir_lowering=False)
    f32, bf16 = mybir.dt.float32, mybir.dt.bfloat16
    f8 = mybir.dt.float8e4

    xT = nc.declare_dram_parameter("xT", [E, BL], bf16, isOutput=False)
    hT = nc.declare_dram_parameter("hT", [H, BL], bf16, isOutput=False)
    cT = nc.declare_dram_parameter("cT", [H, BL], bf16, isOutput=False)
    whwp = nc.declare_dram_parameter("whwp", [NT, 128, E], bf16, isOutput=False)
    bias = nc.declare_dram_parameter("bias", [128, NJI], f32, isOutput=False)
    outT = nc.declare_dram_parameter("outT", [H, BL], bf16, isOutput=True)
    memT = nc.declare_dram_parameter("memT", [H, BL], bf16, isOutput=True)
    if USE_FP8:
        # full-k fp8 weight copies; each gate loads only its first KF k-tiles
        wq8x = nc.declare_dram_parameter("wq8x", [NJS, 128, KT, 128], f8, isOutput=False)
        wq8h = nc.declare_dram_parameter("wq8h", [NJS, 128, KT, 128], f8, isOutput=False)
        wbx = nc.declare_dram_parameter("wbx", [NJS, 128, KB * 128], bf16, isOutput=False)
        wbh = nc.declare_dram_parameter("wbh", [NJS, 128, KB * 128], bf16, isOutput=False)
        xq8 = nc.declare_dram_parameter("xq8", [128, KT, BL], f8, isOutput=False)
        hq8 = nc.declare_dram_parameter("hq8", [128, KT, BL], f8, isOutput=False)
        scl = nc.declare_dram_parameter("scl", [128, 1], f32, isOutput=False)
    else:
        wbx = nc.declare_dram_parameter("wbx", [NJS, 128, E], bf16, isOutput=False)
        wbh = nc.declare_dram_parameter("wbh", [NJS, 128, H], bf16, isOutput=False)

    with tile.TileContext(nc) as tc:
        with (
            tc.tile_pool(name="resident", bufs=1) as resident,
            tc.tile_pool(name="wpool", bufs=4) as wpool,
            tc.tile_pool(name="cpool", bufs=2) as cpool,
            tc.tile_pool(name="psum", bufs=2, space="PSUM") as psum_pool,
            tc.tile_pool(name="gates", bufs=9) as gate_pool,
            tc.tile_pool(name="tmp", bufs=4) as tmp_pool,
            tc.tile_pool(name="outp", bufs=4) as out_pool,
        ):
            def split_dma(dst, src, nsplit, eng=None):
                eng = eng or nc.sync
                n = dst.shape[-1]
                per = n // nsplit
                for q in range(nsplit):
                    sl = slice(q * per, (q + 1) * per)
                    eng.dma_start(dst[:, sl], src[:, sl])

            bias_sb = resident.tile([128, NJI], f32, tag="bias")
            nc.sync.dma_start(bias_sb[:], bias[:])
            if USE_FP8:
                scl_sb = resident.tile([128, 1], f32, tag="scl")
                nc.sync.dma_start(scl_sb[:], scl[:])

            xt_k = [None] + [
                resident.tile([128, BL], bf16, tag=f"xt{k}", name=f"xt{k}")
                for k in range(1, KT)
            ]
            # k=0 is split into two half-tiles so the very first matmuls
            # (bc 0-1) depend on only 256KB of x instead of the full 512KB.
            xt0a = resident.tile([128, BL // 2], bf16, tag="xt0a", name="xt0a")
            xt0b = resident.tile([128, BL // 2], bf16, tag="xt0b", name="xt0b")

            def rhs_x(k, bc):
                if k == 0:
                    t = xt0a if bc < 2 else xt0b
                    return t[:, (bc % 2) * BC : (bc % 2 + 1) * BC]
                return xt_k[k][:, bc * BC : (bc + 1) * BC]
            # ht0/ht1 bf16 are never read when USE_FP8: every gate's h-side
            # k-tiles 0-1 come from hq8 (hwp is x-only), so skip them.
            HT0 = KF if USE_FP8 else 0
            ht_k = [None] * HT0 + [
                resident.tile([128, BL], bf16, tag=f"ht{k}", name=f"ht{k}")
                for k in range(HT0, KT)
            ]
            if USE_FP8:
                xq8_sb = resident.tile([128, KT, BL], f8, tag="xq8")
                hq8_sb = resident.tile([128, KT, BL], f8, tag="hq8")

            # Preloaded weight tiles: three hw_proj tiles (x-only — PE works
            # on these while h streams in) and the first i-gate tile.
            w_hwp0 = wpool.tile([128, E], bf16, tag="w")
            w_hwp1 = wpool.tile([128, E], bf16, tag="w")
            w_hwp2 = wpool.tile([128, E], bf16, tag="w")
            split_dma(xt0a, xT[0:128, : BL // 2], 2, eng=nc.gpsimd)
            split_dma(w_hwp0, whwp[0], 2, eng=nc.gpsimd)
            split_dma(xt0b, xT[0:128, BL // 2 :], 2, eng=nc.gpsimd)
            split_dma(xt_k[1], xT[128:256, :], 4, eng=nc.gpsimd)
            if USE_FP8:
                for q in range(4):
                    sl = slice(q * BL // 4, (q + 1) * BL // 4)
                    nc.gpsimd.dma_start(xq8_sb[:, :, sl], xq8[:, :, sl])
                for q in range(4):
                    sl = slice(q * BL // 4, (q + 1) * BL // 4)
                    nc.gpsimd.dma_start(hq8_sb[:, :, sl], hq8[:, :, sl])
            for k in range(2, 3):
                split_dma(xt_k[k], xT[k * 128 : (k + 1) * 128, :], 4)
            split_dma(w_hwp1, whwp[1], 2)
            split_dma(w_hwp2, whwp[2], 2)

            def load_gate_w(jt, eng=None):
                eng = eng or nc.sync
                if USE_FP8:
                    kf = gate_kf(jt)
                    kb = KT - kf
                    g8x = wpool.tile([128, kf, 128], f8, tag="w8x")
                    eng.dma_start(g8x[:], wq8x[jt][:, :kf, :])
                    gbx = gbh = None
                    if kb:
                        gbx = wpool.tile([128, kb * 128], bf16, tag="wbx",
                                         bufs=3)
                        eng.dma_start(gbx[:], wbx[jt][:, (KB - kb) * 128 :])
                    g8h = wpool.tile([128, kf, 128], f8, tag="w8h")
                    eng.dma_start(g8h[:], wq8h[jt][:, :kf, :])
                    if kb:
                        gbh = wpool.tile([128, kb * 128], bf16, tag="wbh",
                                         bufs=3)
                        eng.dma_start(gbh[:], wbh[jt][:, (KB - kb) * 128 :])
                    return (g8x, gbx, g8h, gbh)
                gbx = wpool.tile([128, E], bf16, tag="wbx")
                eng.dma_start(gbx[:], wbx[jt])
                gbh = wpool.tile([128, H], bf16, tag="wbh")
                eng.dma_start(gbh[:], wbh[jt])
                return (None, gbx, None, gbh)

            w_i0 = load_gate_w(0)
            for k in range(3, KT):
                split_dma(xt_k[k], xT[k * 128 : (k + 1) * 128, :], 4)
            # preload group-0 o/hw gate weights (first gates computed)
            w_o0 = load_gate_w(3 * NT)
            w_hw0 = load_gate_w(4 * NT)
            for k in range(HT0, KT):
                split_dma(ht_k[k], hT[k * 128 : (k + 1) * 128, :], 4)

            def gate_tile(jt, func, w=None, chunk_act=1):
                """Gate proj tile jt (0..NJS-1) -> activated gate (bf16)."""
                if w is None:
                    w = load_gate_w(jt)
                g8x, gbx, g8h, gbh = w
                kf = gate_kf(jt) if USE_FP8 else 0
                ps = psum_pool.tile([128, BL], f32, tag="ps")
                if USE_FP8:
                    for j in range(kf // 2):
                        for bc in range(NBC):
                            nc.tensor.matmul(
                                ps[:, bc * BC : (bc + 1) * BC],
                                g8x[:, 2 * j : 2 * j + 2, :],
                                xq8_sb[:, 2 * j : 2 * j + 2,
                                       bc * BC : (bc + 1) * BC],
                                start=(j == 0), stop=False, perf_mode=DR,
                            )
                for k in range(kf, KT):
                    lhsT = gbx[:, (k - kf) * 128 : (k - kf + 1) * 128]
                    for bc in range(NBC):
                        lo = bc * BC
                        nc.tensor.matmul(
                            ps[:, lo : lo + BC], lhsT, rhs_x(k, bc),
                            start=(not USE_FP8 and k == 0), stop=False,
                        )
                if USE_FP8:
                    for j in range(kf // 2):
                        for bc in range(NBC):
                            nc.tensor.matmul(
                                ps[:, bc * BC : (bc + 1) * BC],
                                g8h[:, 2 * j : 2 * j + 2, :],
                                hq8_sb[:, 2 * j : 2 * j + 2,
                                       bc * BC : (bc + 1) * BC],
                                start=False,
                                stop=(kf == KT and j == kf // 2 - 1),
                                perf_mode=DR,
                            )
                for k in range(kf, KT):
                    lhsT = gbh[:, (k - kf) * 128 : (k - kf + 1) * 128]
                    for bc in range(NBC):
                        lo = bc * BC
                        nc.tensor.matmul(
                            ps[:, lo : lo + BC], lhsT,
                            ht_k[k][:, bc * BC : (bc + 1) * BC],
                            start=False, stop=(k == KT - 1),
                        )
                g = gate_pool.tile([128, BL], bf16, tag="g")
                kw = {"scale": scl_sb[:, 0:1]} if USE_FP8 else {}
                cw = BL // chunk_act
                for a in range(chunk_act):
                    sl = slice(a * cw, (a + 1) * cw)
                    nc.scalar.activation(
                        g[:, sl], ps[:, sl], func,
                        bias=bias_sb[:, jt : jt + 1], **kw
                    )
                return g

            def hwp_tile(t, func=AF.Identity, w_i=None, chunk_act=1,
                         bc0=0, bc1=NBC):
                """hw_proj tile t (x-only, full bf16, unscaled)."""
                jt = 5 * NT + t
                if w_i is None:
                    w_i = wpool.tile([128, E], bf16, tag="w")
                    nc.sync.dma_start(w_i[:], whwp[t])
                width = (bc1 - bc0) * BC
                ps = psum_pool.tile([128, width], f32, tag="ps")
                for k in range(KT):
                    lhsT = w_i[:, k * 128 : (k + 1) * 128]
                    for bc in range(bc0, bc1):
                        lo = (bc - bc0) * BC
                        nc.tensor.matmul(
                            ps[:, lo : lo + BC], lhsT, rhs_x(k, bc),
                            start=(k == 0), stop=(k == KT - 1),
                        )
                g = gate_pool.tile([128, width], bf16, tag="g")
                cw = width // chunk_act
                for a in range(chunk_act):
                    sl = slice(a * cw, (a + 1) * cw)
                    nc.scalar.activation(
                        g[:, sl], ps[:, sl], func, bias=bias_sb[:, jt : jt + 1]
                    )
                return g

            mult, addop, subop = (
                mybir.AluOpType.mult,
                mybir.AluOpType.add,
                mybir.AluOpType.subtract,
            )

            hwp_pre = [
                hwp_tile(0, w_i=w_hwp0),
                hwp_tile(1, w_i=w_hwp1),
                hwp_tile(2, w_i=w_hwp2),
            ]
            for t in range(NT):
                last = t == NT - 1
                if not last:
                    hwp = hwp_pre[t] if t < len(hwp_pre) else hwp_tile(t)
                if t == 0:
                    # group 0: the fp8-only o/hw gates first — they need no
                    # bf16 x/h k-tiles, extending the startup runway before
                    # xt4-7/ht2-7 must have landed
                    o_g = gate_tile(3 * NT + t, AF.Sigmoid, w=w_o0)
                    hw_g = gate_tile(4 * NT + t, AF.Sigmoid, w=w_hw0)
                i_g = gate_tile(t, AF.Sigmoid, w=w_i0 if t == 0 else None)
                m_g = gate_tile(2 * NT + t, AF.Tanh)
                f_g = gate_tile(NT + t, AF.Sigmoid)

                ct = cpool.tile([128, BL], bf16, tag="c")
                nc.sync.dma_start(ct[:], cT[t * 128 : (t + 1) * 128, :])

                t1 = tmp_pool.tile([128, BL], bf16, tag="tmp")
                nc.vector.tensor_tensor(t1[:], i_g[:], m_g[:], mult)
                t2 = tmp_pool.tile([128, BL], bf16, tag="tmp")
                nc.vector.tensor_tensor(t2[:], f_g[:], ct[:], mult)
                mem = out_pool.tile([128, BL], bf16, tag="mem")
                nc.vector.tensor_tensor(mem[:], t1[:], t2[:], addop)
                nc.sync.dma_start(memT[t * 128 : (t + 1) * 128, :], mem[:])

                if t != 0:
                    o_g = gate_tile(3 * NT + t, AF.Sigmoid)

                tmem = tmp_pool.tile([128, BL], bf16, tag="tmp")
                nc.scalar.activation(tmem[:], mem[:], AF.Tanh)
                outp = tmp_pool.tile([128, BL], bf16, tag="tmp")
                nc.vector.tensor_tensor(outp[:], o_g[:], tmem[:], mult)

                if t != 0:
                    hw_g = gate_tile(4 * NT + t, AF.Sigmoid,
                                     chunk_act=4 if last else 1)

                if not last:
                    # out = hwp + hw*(outp - hwp), chunked so the blend
                    # pipelines with the output DMA.
                    u = tmp_pool.tile([128, BL], bf16, tag="tmp")
                    nc.vector.tensor_tensor(u[:], outp[:], hwp[:], subop)
                    for e in range(2):
                        sl = slice(e * (BL // 2), (e + 1) * (BL // 2))
                        v = tmp_pool.tile([128, BL // 2], bf16, tag="v")
                        nc.vector.tensor_tensor(v[:], hw_g[:, sl], u[:, sl], mult)
                        outf = out_pool.tile([128, BL // 2], bf16, tag="out")
                        nc.vector.tensor_tensor(outf[:], v[:], hwp[:, sl], addop)
                        eng = nc.sync if e == 0 else nc.scalar
                        eng.dma_start(outT[t * 128 : (t + 1) * 128, sl], outf[:])
                else:
                    # Last group: hwp computed LAST in per-bc chunks with
                    # fused blend+DMA; final 512 cols split into two 256-col
                    # chunks so the post-matmul tail is minimal.
                    w_hwp_l = wpool.tile([128, E], bf16, tag="w")
                    nc.sync.dma_start(w_hwp_l[:], whwp[t])
                    jt = 5 * NT + t
                    chunks = [(0, BC), (BC, BC), (2 * BC, BC),
                              (3 * BC, BC // 2), (3 * BC + BC // 2, BC // 2)]
                    for ci, (lo, cw) in enumerate(chunks):
                        sl = slice(lo, lo + cw)
                        ps = psum_pool.tile([128, cw], f32, tag="ps")
                        for k in range(KT):
                            nc.tensor.matmul(
                                ps[:],
                                w_hwp_l[:, k * 128 : (k + 1) * 128],
                                rhs_x(k, lo // BC)[:, lo % BC : lo % BC + cw],
                                start=(k == 0),
                                stop=(k == KT - 1),
                            )
                        hwp_c = tmp_pool.tile([128, cw], bf16, tag="v")
                        nc.scalar.activation(
                            hwp_c[:], ps[:], AF.Identity,
                            bias=bias_sb[:, jt : jt + 1],
                        )
                        # alternate blend lanes vector/gpsimd so the final
                        # chunks' element-wise chains run in parallel
                        ve = nc.vector if ci % 2 == 0 else nc.gpsimd
                        u = tmp_pool.tile([128, cw], bf16, tag="v")
                        ve.tensor_tensor(u[:], outp[:, sl], hwp_c[:], subop)
                        v = tmp_pool.tile([128, cw], bf16, tag="v")
                        ve.tensor_tensor(v[:], hw_g[:, sl], u[:], mult)
                        outf = out_pool.tile([128, cw], bf16, tag="out")
                        ve.tensor_tensor(outf[:], v[:], hwp_c[:], addop)
                        eng = nc.sync if ci % 2 == 0 else nc.scalar
                        eng.dma_start(outT[t * 128 : (t + 1) * 128, sl], outf[:])

    nc.compile()
    return nc


_NC_CACHE = None


def _get_nc():
    global _NC_CACHE
    if _NC_CACHE is None:
        _NC_CACHE = build_nc()
    return _NC_CACHE


def _pack_weights(W, njt):
    # W [njt*128 j, K e] -> [njt, 128 p, K] with [jt, p, k*128+m] = W[jt*128+m, k*128+p]
    W = np.asarray(W, np.float32)
    K = W.shape[1]
    kt = K // 128
    return np.ascontiguousarray(
        W.reshape(njt, 128, kt, 128).transpose(0, 3, 2, 1).reshape(njt, 128, K)
    )


def _q8(a, sc):
    return np.clip(np.asarray(a, np.float32) * sc, -240.0, 240.0).astype(F8E4)


def prepare_in_maps(x, h, c, Wi, bi, Ws, bs):
    x = np.asarray(x, np.float32)
    h = np.asarray(h, np.float32)
    Wi = np.asarray(Wi, np.float32)
    Ws = np.asarray(Ws, np.float32)
    Wg = Wi[: 5 * H]

    bias_comb = np.concatenate(
        [np.asarray(bi[: 5 * H], np.float32) + np.asarray(bs, np.float32),
         np.asarray(bi[5 * H :], np.float32)]
    )
    bias_pack = np.ascontiguousarray(bias_comb.reshape(NJI, 128).T).astype(np.float32)
    whwp_p = _pack_weights(Wi[5 * H :], NT).astype(BF16)

    common = {"bias": bias_pack, "whwp": whwp_p}
    kf = KF * 128
    if USE_FP8:
        sW = np.float32(224.0 / max(np.abs(Wg).max(), np.abs(Ws).max()))
        sA = np.float32(224.0 / max(np.abs(x).max(), np.abs(h).max()))
        S = np.float32(sW * sA)
        # [jt, m, j, p] -> [jt, p, j, m]
        wq8x_p = _q8(Wg.reshape(NJS, 128, KT, 128).transpose(0, 3, 2, 1), sW)
        wq8h_p = _q8(Ws.reshape(NJS, 128, KT, 128).transpose(0, 3, 2, 1), sW)
        wbx_p = np.ascontiguousarray(
            _pack_weights(Wg * S, NJS)[:, :, kf:]).astype(BF16)
        wbh_p = np.ascontiguousarray(
            _pack_weights(Ws * S, NJS)[:, :, kf:]).astype(BF16)
        common.update(
            wq8x=np.ascontiguousarray(wq8x_p),
            wq8h=np.ascontiguousarray(wq8h_p),
            wbx=wbx_p,
            wbh=wbh_p,
            scl=np.full((128, 1), 1.0 / S, np.float32),
        )
    else:
        common.update(
            wbx=_pack_weights(Wg, NJS).astype(BF16),
            wbh=_pack_weights(Ws, NJS).astype(BF16),
        )

    in_maps = []
    for i in range(N_CORES):
        s = slice(i * BL, (i + 1) * BL)
        m = {
            "xT": np.ascontiguousarray(x[s].T).astype(BF16),
            "hT": np.ascontiguousarray(h[s].T).astype(BF16),
            "cT": np.ascontiguousarray(np.asarray(c[s], np.float32).T).astype(BF16),
        }
        if USE_FP8:
            # [p, j, b] with value in[b, j*128+p]
            m["xq8"] = np.ascontiguousarray(
                _q8(x[s].T.reshape(KT, 128, BL).transpose(1, 0, 2), sA))
            m["hq8"] = np.ascontiguousarray(
                _q8(h[s].T.reshape(KT, 128, BL).transpose(1, 0, 2), sA))
        m.update(common)
        in_maps.append(m)
    return in_maps


def run(in_maps, trace=False):
    nc = _get_nc()
    res = run_bass_kernel_spmd(nc, in_maps, core_ids=list(range(N_CORES)), trace=trace)
    out = np.empty((B, H), np.float32)
    mem = np.empty((B, H), np.float32)
    for i in range(N_CORES):
        s = slice(i * BL, (i + 1) * BL)
        out[s] = res.results[i]["outT"].astype(np.float32).T
        mem[s] = res.results[i]["memT"].astype(np.float32).T
    return (out, mem), res


def kernel(x, h, c, Wi, bi, Ws, bs):
    in_maps = prepare_in_maps(x, h, c, Wi, bi, Ws, bs)
    (out, mem), _ = run(in_maps, trace=False)
    return out, mem
